# revision 23
# baseline (speedup 1.0000x reference)
"""Trainium2 Bass kernel for nn_Encoding_layer (highway stack + pairwise MLP
attention + fuse gates).

Sharding: data-parallel over batch B=16 across 8 NeuronCores (2 batches per
core); all dense weights replicated. No collectives.

FP8 (TRN e4m3, DoubleRow = 2 k-tiles/instruction) on highway layer 0, the
pairwise scores, the attention numerator/denominator, and the att-half of
the fuse gates; small weights pre-scaled by 64 into e4m3 normal range,
de-scaled for free via activation `scale`. Kept bf16: highway layer 1 (DVE
ops with fp8 inputs run ~8x slow, so an fp8 x1 costs more elementwise than
the matmul saves) and the x0-half of the fuse gates (x0 carries unit-scale
values whose e4m3 noise would land directly on the output; att values are
~30x smaller so their fp8 noise is negligible). Both fuse halves are x64 so
they can share one PSUM accumulation.

Layouts (n = 2 batches x 1024 = 2048 token-columns):
  xTh/x1T/x2T [128,KU,n] bf16 transposed activations [u%128, u/128, n];
  xq/x2q/w3xq/attq [128,KU,n] fp8; xOq [128,NT,U] fp8 row-major; attT bf16.
Pipeline: input transposes stream under the input DMAs and highway L0 rides
behind them tile-group by tile-group (weight DMAs are issued after the
second input group so they land just as L0 becomes runnable). Highway L1
carries one-unit-delayed fp8 cast + w3x slices, fp8 row-major transposes
per finished token-slice, and the fuse-gate weight drip, all placed in
engine slack. s1 and s2 come from one two-row GEMM (w1|w2 packed in the
stationary) plus a 1-row PE transpose, avoiding 32 stationary reloads.
Attention: S^T[j,i]=64*s3 fp8; exp on ACT (scale 1/64, bias s2[j]); the
per-column s1[i]+ab cancels in softmax so relu becomes a clamp against
th[i]=exp(-(s1+ab)); numerator/denominator are fp8 DoubleRow matmuls over
M^T j-tile pairs (per-pair ehq buffers); the denominator accumulates into
row 0 of the broadcast PSUM tile (saves a bank, enabling double-buffered
broadcasts), the next unit's threshold broadcast is prefetched before the
current unit's drains, and numerator banks drain to bf16 immediately with
normalization deferred off the critical path. Phase E preloads 64x biases
into PSUM via DVE (start=False chains) and casts attq one tile ahead.
"""

import numpy as np

B, L, U, H = 16, 1024, 512, 2
NCORES = 8
BPC = B // NCORES          # batches per core
N = BPC * L                # token columns per core
KU = U // 128              # 4  u-tiles
NT = N // 128              # 16 row-tiles per core
NS = N // 512              # 4  512-wide column slices per core
JT = L // 128              # 8  j-tiles per batch
IH = L // 512              # 2  i-halves per batch
WS = 64.0                  # fp8 weight pre-scale
WSI = 1.0 / WS


def build_nc():
    import concourse.bacc as bacc
    import concourse.tile as tile
    from concourse import mybir
    from concourse.masks import make_identity

    F32 = mybir.dt.float32
    F32R = mybir.dt.float32r
    BF16 = mybir.dt.bfloat16
    FP8 = mybir.dt.float8e4
    AF = mybir.ActivationFunctionType
    OP = mybir.AluOpType
    PM = mybir.MatmulPerfMode

    nc = bacc.Bacc("TRN2", target_bir_lowering=False, debug=False,
                   num_devices=NCORES)

    x_in = nc.dram_tensor("inputs", [BPC, L, U], F32, kind="ExternalInput").ap()
    tW = nc.dram_tensor("tW", [H, U, U], F32, kind="ExternalInput").ap()
    tb = nc.dram_tensor("tb", [H, U], F32, kind="ExternalInput").ap()
    cW = nc.dram_tensor("cW", [H, U, U], F32, kind="ExternalInput").ap()
    cb = nc.dram_tensor("cb", [H, U], F32, kind="ExternalInput").ap()
    aW = nc.dram_tensor("aW", [3 * U], F32, kind="ExternalInput").ap()
    ab = nc.dram_tensor("ab", [1], F32, kind="ExternalInput").ap()
    frW = nc.dram_tensor("frW", [2 * U, U], F32, kind="ExternalInput").ap()
    frb = nc.dram_tensor("frb", [U], F32, kind="ExternalInput").ap()
    ffW = nc.dram_tensor("ffW", [2 * U, U], F32, kind="ExternalInput").ap()
    ffb = nc.dram_tensor("ffb", [U], F32, kind="ExternalInput").ap()
    out = nc.dram_tensor("out", [BPC, L, U], F32, kind="ExternalOutput").ap()

    xv = x_in.flatten_outer_dims().rearrange("(t p) u -> t p u", p=128)
    outv = out.flatten_outer_dims().rearrange("(t p) u -> t p u", p=128)

    with tile.TileContext(nc) as tc:
        with tc.tile_pool(name="pers", bufs=1) as pers:
            # ---- persistent SBUF tensors ----
            xTh = pers.tile([128, KU, N], BF16, tag="xTh")    # inputs^T bf16
            xq = pers.tile([128, KU, N], FP8, tag="xq")       # inputs^T fp8
            x1T = pers.tile([128, KU, N], BF16, tag="x1T")
            x2T = pers.tile([128, KU, N], BF16, tag="x2T")
            x2q = pers.tile([128, KU, N], FP8, tag="x2q")
            w3xq = pers.tile([128, KU, N], FP8, tag="w3xq")   # (64*w3)*x^T
            attT = pers.tile([128, KU, N], BF16, tag="attT")
            attq = pers.tile([128, KU, N], FP8, tag="attq")
            xOq = pers.tile([128, NT, U], FP8, tag="xOq")     # x2 row-major
            tWq = pers.tile([128, KU, U], FP8, tag="tWq")     # 64*tW[0]
            cWq = pers.tile([128, KU, U], FP8, tag="cWq")     # 64*cW[0]
            tWh = pers.tile([128, KU, U], BF16, tag="tWh")    # tW[1]
            cWh = pers.tile([128, KU, U], BF16, tag="cWh")    # cW[1]
            ffWh = pers.tile([128, KU, U], BF16, tag="ffWh")  # 64*ffW[:U]
            frWh = pers.tile([128, KU, U], BF16, tag="frWh")  # 64*frW[:U]
            ffWq = pers.tile([128, KU, U], FP8, tag="ffWq")   # 64*ffW[U:]
            frWq = pers.tile([128, KU, U], FP8, tag="frWq")   # 64*frW[U:]
            tbsb = pers.tile([128, H, KU], F32, tag="tbsb")
            cbsb = pers.tile([128, H, KU], F32, tag="cbsb")
            awsb = pers.tile([128, 12], F32, tag="awsb")      # w1|w2|w3 cols
            # k-pair stride of DoubleRow operands must be 16B-aligned:
            # w1 and w2 are packed as two 64x-scaled columns of one
            # 16-wide stationary so s1 and s2 come from a single GEMM
            w12q = pers.tile([128, KU, 16], FP8, tag="w12q")
            aw3s = pers.tile([128, KU], F32, tag="aw3s")      # 64*w3
            ab_sb = pers.tile([1, 1], F32, tag="ab_sb")
            nab_sb = pers.tile([1, 1], F32, tag="nab_sb")
            ffb_h = pers.tile([1, U], BF16, tag="ffb_h")      # 64*ffb
            frb_h = pers.tile([1, U], BF16, tag="frb_h")      # 64*frb
            fbbc = pers.tile([128, U], F32, tag="fbbc")       # 64*ffb bcast
            rbbc = pers.tile([128, U], F32, tag="rbbc")       # 64*frb bcast
            thr = pers.tile([1, N], BF16, tag="thr")   # exp(-(s1+ab))
            s2r = pers.tile([1, N], BF16, tag="s2r")   # s2 row-major
            s2f = pers.tile([128, NT], F32, tag="s2f")
            ones_row = pers.tile([1, 128], BF16, tag="ones_row")
            onesq = pers.tile([128, 2, 16], FP8, tag="onesq")
            ident = pers.tile([128, 128], BF16, tag="ident")
            identq = pers.tile([128, 128], FP8, tag="identq")
            identf = pers.tile([128, 128], F32, tag="identf")

            nc.vector.memset(ones_row, 1.0)
            nc.vector.memset(onesq, 1.0)
            make_identity(nc, ident)
            make_identity(nc, identf)
            nc.vector.tensor_copy(identq, ident)

            fWv = ffW.rearrange("(k p) m -> k p m", p=128)
            rWv = frW.rearrange("(k p) m -> k p m", p=128)
            # x0-half chunks in bf16 (x64), att-half in fp8 (x64)
            fuse_bf = [(fWv, ffWh, k) for k in range(KU)] + \
                      [(rWv, frWh, k) for k in range(KU)]
            fuse_q = [(fWv, ffWq, k) for k in range(KU)] + \
                     [(rWv, frWq, k) for k in range(KU)]

            with tc.tile_pool(name="stg", bufs=6) as stg, \
                 tc.tile_pool(name="stgw", bufs=6) as stgw, \
                 tc.tile_pool(name="hws", bufs=4) as hws, \
                 tc.tile_pool(name="stgf", bufs=4) as stgf:

                def drip_bf(ci):
                    wv_, wdst_, k_ = fuse_bf[ci]
                    wsf = stgf.tile([128, U], F32, tag="wsf",
                                    name=f"wsf_b{ci}")
                    nc.sync.dma_start(wsf, wv_[k_])
                    nc.vector.tensor_scalar_mul(wdst_[:, k_, :], wsf, WS)

                def drip_q(ci):
                    wv_, wdst_, k_ = fuse_q[ci]
                    wsf = stgf.tile([128, U], F32, tag="wsf",
                                    name=f"wsf_q{ci}")
                    nc.sync.dma_start(wsf, wv_[KU + k_])
                    nc.scalar.activation(wdst_[:, k_, :], wsf, AF.Copy,
                                         scale=WS)

                with tc.tile_pool(name="hwp", bufs=2, space="PSUM") as hwp:

                    def hw_unit(l, t, m):
                        xin = xq if l == 0 else x1T
                        xinb = xTh if l == 0 else x1T
                        xout = x1T if l == 0 else x2T
                        nsl = slice(t * 512, (t + 1) * 512)
                        msl = slice(m * 128, (m + 1) * 128)
                        unit = t * KU + m
                        pt = hwp.tile([128, 512], F32, tag="pt")
                        pc = hwp.tile([128, 512], F32, tag="pc")
                        if l == 0:
                            for k2 in range(2):
                                ksl = slice(2 * k2, 2 * k2 + 2)
                                nc.tensor.matmul(
                                    pt, tWq[:, ksl, msl], xin[:, ksl, nsl],
                                    start=(k2 == 0), stop=(k2 == 1),
                                    perf_mode=PM.DoubleRow)
                            for k2 in range(2):
                                ksl = slice(2 * k2, 2 * k2 + 2)
                                nc.tensor.matmul(
                                    pc, cWq[:, ksl, msl], xin[:, ksl, nsl],
                                    start=(k2 == 0), stop=(k2 == 1),
                                    perf_mode=PM.DoubleRow)
                            sc = WSI
                        else:
                            for k in range(KU):
                                nc.tensor.matmul(
                                    pt, tWh[:, k, msl], xin[:, k, nsl],
                                    start=(k == 0), stop=(k == KU - 1))
                            for k in range(KU):
                                nc.tensor.matmul(
                                    pc, cWh[:, k, msl], xin[:, k, nsl],
                                    start=(k == 0), stop=(k == KU - 1))
                            sc = 1.0
                        th = hws.tile([128, 512], BF16, tag="th")
                        ch = hws.tile([128, 512], BF16, tag="ch")
                        nc.scalar.activation(
                            th, pt, AF.Relu, bias=tbsb[:, l, m:m + 1],
                            scale=sc)
                        nc.scalar.activation(
                            ch, pc, AF.Sigmoid, bias=cbsb[:, l, m:m + 1],
                            scale=sc)
                        dh = hws.tile([128, 512], BF16, tag="dh")
                        nc.vector.tensor_tensor(
                            dh, th, xinb[:, m, nsl], op=OP.subtract)
                        mh = hws.tile([128, 512], BF16, tag="mh")
                        nc.vector.tensor_tensor(mh, ch, dh, op=OP.mult)
                        nc.gpsimd.tensor_tensor(
                            xout[:, m, nsl], xinb[:, m, nsl], mh, op=OP.add)
                        if l == 0:
                            if unit % 2 == 0:
                                drip_bf(unit // 2)
                        else:
                            if unit % 2 == 0:
                                drip_q(unit // 2)

                    # ===== Phase A + L0 + L1 software-pipelined ============
                    # psum: hwp 4 + ptA 2 + pcp 2 = 8 banks
                    with tc.tile_pool(name="ptA", bufs=1,
                                      space="PSUM") as ptA, \
                         tc.tile_pool(name="pcp", bufs=2,
                                      space="PSUM") as pcp:

                        def l1_delayed(t, m):
                            nsl = slice(t * 512, (t + 1) * 512)
                            nc.vector.tensor_copy(x2q[:, m, nsl],
                                                  x2T[:, m, nsl])
                            nc.scalar.activation(
                                w3xq[:, m, nsl], x2q[:, m, nsl], AF.Copy,
                                scale=aw3s[:, m:m + 1])

                        def xo_jt(jt):
                            # bf16 transpose straight from x2T (no wait on
                            # the fp8 cast); DVE drains to fp8
                            ptr = pcp.tile([128, KU, 128], BF16,
                                           tag="ptr", name=f"ptr_{jt}")
                            for k in range(KU):
                                nc.tensor.transpose(
                                    ptr[:, k, :],
                                    x2T[:, k, jt * 128:(jt + 1) * 128],
                                    ident)
                            nc.vector.tensor_copy(xOq[:, jt, :], ptr)

                        def l1_unit(t, m):
                            hw_unit(1, t, m)
                            unit = t * KU + m
                            if unit >= 1:
                                pm_ = (unit - 1) % 4
                                pt_ = (unit - 1) // 4
                                l1_delayed(pt_, pm_)
                            if t >= 1:
                                xo_jt(4 * (t - 1) + m)

                        def transposes(tg):
                            xss = []
                            for tt in range(4):
                                t = tg * 4 + tt
                                xs = stg.tile([128, U], F32, tag="xs",
                                              name=f"xs_{t}")
                                nc.sync.dma_start(xs[:, 0:256],
                                                  xv[t][:, 0:256])
                                nc.sync.dma_start(xs[:, 256:512],
                                                  xv[t][:, 256:512])
                                xss.append(xs)
                            tsl = slice(tg * 512, (tg + 1) * 512)
                            for k in range(KU):
                                ptk = ptA.tile([128, 512], F32,
                                               tag=f"ptk{k % 2}",
                                               name=f"ptk_{tg}_{k}")
                                for tt in range(4):
                                    nc.tensor.transpose(
                                        ptk[:, tt * 128:(tt + 1) * 128],
                                        xss[tt][:, k * 128:(k + 1) * 128],
                                        identf)
                                nc.vector.tensor_copy(xTh[:, k, tsl], ptk)
                                if k % 2 == 0:
                                    nc.vector.tensor_copy(xq[:, k, tsl],
                                                          ptk)
                                else:
                                    nc.scalar.activation(xq[:, k, tsl],
                                                         ptk, AF.Copy)

                        def emit_hw_weights():
                            for l in range(H):
                                for wi, wsrc in ((0, tW), (1, cW)):
                                    wdst = ((tWq, cWq), (tWh, cWh))[l][wi]
                                    wv = wsrc[l].rearrange(
                                        "(k p) m -> k p m", p=128)
                                    for k in range(KU):
                                        ws = stgw.tile(
                                            [128, U], F32, tag="ws",
                                            name=f"ws_{l}_{wi}_{k}")
                                        nc.sync.dma_start(ws[:, 0:256],
                                                          wv[k][:, 0:256])
                                        nc.sync.dma_start(ws[:, 256:512],
                                                          wv[k][:, 256:512])
                                        if l == 0:
                                            if k % 2 == 0:
                                                nc.vector.tensor_scalar_mul(
                                                    wdst[:, k, :], ws, WS)
                                            else:
                                                nc.scalar.activation(
                                                    wdst[:, k, :], ws,
                                                    AF.Copy, scale=WS)
                                        else:
                                            if k % 2 == 0:
                                                nc.vector.tensor_copy(
                                                    wdst[:, k, :], ws)
                                            else:
                                                nc.scalar.copy(
                                                    wdst[:, k, :], ws)

                        def emit_smalls():
                            nc.sync.dma_start(
                                tbsb, tb.rearrange("l (m p) -> p l m",
                                                   p=128))
                            nc.sync.dma_start(
                                cbsb, cb.rearrange("l (m p) -> p l m",
                                                   p=128))
                            nc.sync.dma_start(
                                awsb,
                                aW.rearrange("(w m p) -> p (w m)",
                                             p=128, w=3))
                            nc.vector.tensor_scalar_mul(w12q[:, :, 0],
                                                        awsb[:, 0:KU], WS)
                            nc.vector.tensor_scalar_mul(w12q[:, :, 8],
                                                        awsb[:, KU:2 * KU],
                                                        WS)
                            nc.vector.tensor_scalar_mul(
                                aw3s, awsb[:, 2 * KU:3 * KU], WS)
                            nc.sync.dma_start(ab_sb, ab[None, :])
                            nc.scalar.mul(nab_sb, ab_sb, -1.0)
                            fb = stg.tile([1, U], F32, tag="fb")
                            nc.sync.dma_start(fb, ffb[None, :])
                            nc.vector.tensor_scalar_mul(ffb_h, fb, WS)
                            fb2 = stg.tile([1, U], F32, tag="fb")
                            nc.sync.dma_start(fb2, frb[None, :])
                            nc.vector.tensor_scalar_mul(frb_h, fb2, WS)

                        # input DMAs first (first-tile latency gates the
                        # transposes); params + weights queue behind
                        warm = [ptA.tile([128, 512], F32, tag=f"ptk{k}",
                                         name=f"warm_{k}")
                                for k in range(2)]
                        for i in range(40):
                            nc.tensor.matmul(warm[i % 2][:, 0:128], ident,
                                             ident, start=True, stop=True)
                        transposes(0)
                        transposes(1)
                        emit_smalls()
                        emit_hw_weights()
                        for m in range(KU):
                            hw_unit(0, 0, m)
                        transposes(2)
                        for m in range(KU):
                            hw_unit(0, 1, m)
                        for m in range(KU):
                            l1_unit(0, m)
                        transposes(3)
                        for m in range(KU):
                            hw_unit(0, 2, m)
                        for m in range(KU):
                            l1_unit(1, m)
                        for m in range(KU):
                            hw_unit(0, 3, m)
                        for m in range(KU):
                            l1_unit(2, m)
                        for m in range(KU):
                            l1_unit(3, m)
                        l1_delayed(3, 3)
                        for jt in range(12, 16):
                            xo_jt(jt)

                # ============= mini-C: s1/s2 + bias broadcasts =============
                with tc.tile_pool(name="pcp1", bufs=1, space="PSUM") as pcp1:
                    for t in range(NS):
                        tsl = slice(t * 512, (t + 1) * 512)
                        ps1 = pcp1.tile([1, 512], F32, tag="ps1",
                                        name=f"ps1_{t}")
                        for k2 in range(2):
                            ksl = slice(2 * k2, 2 * k2 + 2)
                            nc.tensor.matmul(
                                ps1, w12q[:, ksl, 0:1], x2q[:, ksl, tsl],
                                start=(k2 == 0), stop=(k2 == 1),
                                perf_mode=PM.DoubleRow)
                        nc.scalar.activation(thr[:, tsl], ps1,
                                             AF.Exp, bias=nab_sb, scale=-WSI)
                        ps2 = pcp1.tile([1, 512], F32, tag="ps2",
                                        name=f"ps2_{t}")
                        for k2 in range(2):
                            ksl = slice(2 * k2, 2 * k2 + 2)
                            nc.tensor.matmul(
                                ps2, w12q[:, ksl, 8:9], x2q[:, ksl, tsl],
                                start=(k2 == 0), stop=(k2 == 1),
                                perf_mode=PM.DoubleRow)
                        nc.vector.tensor_scalar_mul(s2r[:, tsl], ps2, WSI)
                    s2tp = pcp1.tile([128, NT, 2], BF16, tag="s2tp")
                    for jt in range(NT):
                        nc.tensor.transpose(
                            s2tp[:, jt, 0:1],
                            s2r[:, jt * 128:(jt + 1) * 128],
                            ones_row[:, 0:1])
                    nc.vector.tensor_copy(s2f, s2tp[:, :, 0])
                    pfb = pcp1.tile([128, U], F32, tag="pfb")
                    nc.tensor.matmul(pfb, ones_row, ffb_h,
                                     start=True, stop=True)
                    nc.vector.tensor_copy(fbbc, pfb)
                    prb = pcp1.tile([128, U], F32, tag="prb")
                    nc.tensor.matmul(prb, ones_row, frb_h,
                                     start=True, stop=True)
                    nc.vector.tensor_copy(rbbc, prb)

            # ============= Phase D: pairwise softmax attention (fp8) =======
            units = [(b, h) for b in range(BPC) for h in range(IH)]
            with tc.tile_pool(name="pdn", bufs=4, space="PSUM") as pdn, \
                 tc.tile_pool(name="pds", bufs=2, space="PSUM") as pds, \
                 tc.tile_pool(name="pbc", bufs=2, space="PSUM") as pbc, \
                 tc.tile_pool(name="dsb", bufs=4) as dsb:

                def emit_bcast(u):
                    # threshold broadcast for unit u; the denominator later
                    # accumulates into row 0 of the same psum tile
                    b, h = units[u]
                    isl = slice(b * L + h * 512, b * L + (h + 1) * 512)
                    pb = pbc.tile([128, 512], F32, tag="pb",
                                  name=f"pb1_{u}")
                    nc.tensor.matmul(pb, ones_row, thr[:, isl],
                                     start=True, stop=True)
                    tb_ = dsb.tile([128, 512], BF16, tag="thbc",
                                   name=f"thbc_{u}")
                    nc.scalar.copy(tb_, pb)
                    return pb, tb_

                nxt = emit_bcast(0)
                for u, (b, h) in enumerate(units):
                    pb1, thbc = nxt
                    pr = pb1[0:1, :]
                    isl = slice(b * L + h * 512, b * L + (h + 1) * 512)
                    pn = [pdn.tile([128, 512], F32, tag="pn",
                                   name=f"pn_{u}_{du}")
                          for du in range(KU)]
                    for jt in range(JT):
                        jg = b * JT + jt
                        jsl = slice(b * L + jt * 128, b * L + (jt + 1) * 128)
                        if jt % 2 == 0:
                            ehq = dsb.tile([128, 2, 512], FP8, tag="ehq",
                                           name=f"ehq_{u}_{jt // 2}")
                        ps = pds.tile([128, 512], F32, tag="ps")
                        for k2 in range(2):
                            ksl = slice(2 * k2, 2 * k2 + 2)
                            nc.tensor.matmul(ps, w3xq[:, ksl, jsl],
                                             x2q[:, ksl, isl],
                                             start=(k2 == 0), stop=(k2 == 1),
                                             perf_mode=PM.DoubleRow)
                        eh = dsb.tile([128, 512], BF16, tag="eh")
                        nc.scalar.activation(eh, ps, AF.Exp,
                                             bias=s2f[:, jg:jg + 1],
                                             scale=WSI)
                        nc.vector.tensor_tensor(ehq[:, jt % 2, :],
                                                eh, thbc, op=OP.max)
                        if jt % 2 == 1:
                            jg0 = jg - 1
                            pair = jt // 2
                            for du in range(KU):
                                nc.tensor.matmul(
                                    pn[du],
                                    xOq[:, jg0:jg0 + 2,
                                        du * 128:(du + 1) * 128],
                                    ehq,
                                    start=(pair == 0),
                                    stop=(pair == JT // 2 - 1),
                                    perf_mode=PM.DoubleRow)
                            nc.tensor.matmul(pr, onesq[:, :, 0:1], ehq,
                                             start=(pair == 0),
                                             stop=(pair == JT // 2 - 1),
                                             perf_mode=PM.DoubleRow)
                    rec = dsb.tile([1, 512], F32, tag="rec")
                    nc.vector.reciprocal_approx_fast(rec, pr)
                    rech = dsb.tile([1, 512], BF16, tag="rech")
                    nc.vector.tensor_copy(rech, rec)
                    if u + 1 < len(units):
                        nxt = emit_bcast(u + 1)
                    # drain numerator banks promptly; normalize off-path
                    pnh = [dsb.tile([128, 512], BF16, tag="pnh",
                                    name=f"pnh_{u}_{du}")
                           for du in range(KU)]
                    for du in range(KU):
                        if du % 2 == 0:
                            nc.scalar.copy(pnh[du], pn[du])
                        else:
                            nc.vector.tensor_copy(pnh[du], pn[du])
                    rbc = dsb.tile([128, 512], BF16, tag="rbc")
                    pb2 = pbc.tile([128, 512], F32, tag="pb",
                                   name=f"pb2_{u}")
                    nc.tensor.matmul(pb2, ones_row, rech,
                                     start=True, stop=True)
                    nc.scalar.copy(rbc, pb2)
                    for du in (0, 3):
                        nc.vector.tensor_tensor(
                            attT[:, du, isl], pnh[du], rbc, op=OP.mult)
                    for du in (1, 2):
                        nc.gpsimd.tensor_tensor(
                            attT[:, du, isl], pnh[du], rbc, op=OP.mult)

            # ==== Phase E: fuse gates (x0 bf16 + att fp8, both x64) ========
            with tc.tile_pool(name="pep", bufs=2, space="PSUM") as pep, \
                 tc.tile_pool(name="esb", bufs=3) as esb:
                nc.vector.tensor_copy(attq[:, :, 0:128], attT[:, :, 0:128])
                for mt in range(NT):
                    msl = slice(mt * 128, (mt + 1) * 128)
                    if mt + 1 < NT:
                        nsl = slice((mt + 1) * 128, (mt + 2) * 128)
                        nc.vector.tensor_copy(attq[:, :, nsl],
                                              attT[:, :, nsl])
                    x0t = esb.tile([128, U], F32, tag="x0t")
                    nc.sync.dma_start(x0t, xv[mt])
                    pz = pep.tile([128, 512], F32, tag="pz")
                    pr2 = pep.tile([128, 512], F32, tag="pr2")
                    nc.vector.tensor_copy(pz, fbbc)
                    nc.scalar.copy(pr2, rbbc)
                    for k in range(KU):
                        lhsT = xTh[:, k, msl]
                        nc.tensor.matmul(pz, lhsT, ffWh[:, k, :],
                                         start=False, stop=False)
                        nc.tensor.matmul(pr2, lhsT, frWh[:, k, :],
                                         start=False, stop=False)
                    for i in range(KU // 2):
                        ksl = slice(2 * i, 2 * i + 2)
                        lhsT = attq[:, ksl, msl]
                        nc.tensor.matmul(pz, lhsT, ffWq[:, ksl, :],
                                         start=False,
                                         stop=(i == KU // 2 - 1),
                                         perf_mode=PM.DoubleRow)
                        nc.tensor.matmul(pr2, lhsT, frWq[:, ksl, :],
                                         start=False,
                                         stop=(i == KU // 2 - 1),
                                         perf_mode=PM.DoubleRow)
                    zh = esb.tile([128, U], BF16, tag="zh")
                    rh = esb.tile([128, U], BF16, tag="rh")
                    q = esb.tile([128, U], F32, tag="q")
                    p2 = esb.tile([128, U], F32, tag="p2")
                    ot = esb.tile([128, U], F32, tag="ot")
                    if mt == NT - 1:
                        # last unit sets the kernel tail: shorten its
                        # serial chain by splitting across engines
                        hU = U // 2
                        nc.scalar.activation(zh, pz, AF.Sigmoid, scale=WSI)
                        nc.scalar.square(q, zh)
                        nc.scalar.activation(rh, pr2, AF.Sigmoid, scale=WSI)
                        nc.vector.tensor_tensor(p2[:, :hU], rh[:, :hU],
                                                x0t[:, :hU], op=OP.mult)
                        nc.gpsimd.tensor_tensor(p2[:, hU:], rh[:, hU:],
                                                x0t[:, hU:], op=OP.mult)
                        nc.vector.tensor_tensor(ot[:, :hU], q[:, :hU],
                                                p2[:, :hU], op=OP.add)
                        nc.gpsimd.tensor_tensor(ot[:, hU:], q[:, hU:],
                                                p2[:, hU:], op=OP.add)
                    else:
                        nc.scalar.activation(zh, pz, AF.Sigmoid, scale=WSI)
                        nc.scalar.activation(rh, pr2, AF.Sigmoid, scale=WSI)
                        nc.gpsimd.tensor_tensor(q, zh, zh, op=OP.mult)
                        if mt % 2 == 0:
                            nc.vector.tensor_tensor(p2, rh, x0t, op=OP.mult)
                            nc.vector.tensor_tensor(ot, q, p2, op=OP.add)
                        else:
                            nc.gpsimd.tensor_tensor(p2, rh, x0t, op=OP.mult)
                            nc.gpsimd.tensor_tensor(ot, q, p2, op=OP.add)
                    nc.sync.dma_start(outv[mt], ot)

    nc.compile()
    return nc


_NC_CACHE = None


def _get_nc():
    global _NC_CACHE
    if _NC_CACHE is None:
        _NC_CACHE = build_nc()
    return _NC_CACHE


def kernel(**inputs) -> np.ndarray:
    from concourse.bass_utils import run_bass_kernel_spmd

    nc = _get_nc()
    full = {k: np.ascontiguousarray(np.asarray(v, dtype=np.float32))
            for k, v in inputs.items()}
    in_maps = []
    for c in range(NCORES):
        m = dict(full)
        m["inputs"] = np.ascontiguousarray(
            full["inputs"][c * BPC:(c + 1) * BPC])
        in_maps.append(m)
    res = run_bass_kernel_spmd(nc, in_maps, core_ids=list(range(NCORES)))
    return np.concatenate([res.results[c]["out"] for c in range(NCORES)],
                          axis=0)


# revision 24
# speedup vs baseline: 1.0262x; 1.0262x over previous
"""Trainium2 Bass kernel for nn_Encoding_layer (highway stack + pairwise MLP
attention + fuse gates).

Sharding: data-parallel over batch B=16 across 8 NeuronCores (2 batches per
core); all dense weights replicated. No collectives.

FP8 (TRN e4m3, DoubleRow = 2 k-tiles/instruction) on highway layer 0, the
pairwise scores, the attention numerator/denominator, and the att-half of
the fuse gates; small weights pre-scaled by 64 into e4m3 normal range,
de-scaled for free via activation `scale`. Kept bf16: highway layer 1 (DVE
ops with fp8 inputs run ~8x slow, so an fp8 x1 costs more elementwise than
the matmul saves) and the x0-half of the fuse gates (x0 carries unit-scale
values whose e4m3 noise would land directly on the output; att values are
~30x smaller so their fp8 noise is negligible). Both fuse halves are x64 so
they can share one PSUM accumulation.

Layouts (n = 2 batches x 1024 = 2048 token-columns):
  xTh/x1T/x2T [128,KU,n] bf16 transposed activations [u%128, u/128, n];
  xq/x2q/w3xq/attq [128,KU,n] fp8; xOq [128,NT,U] fp8 row-major; attT bf16.
Pipeline: input transposes stream under the input DMAs and highway L0 rides
behind them tile-group by tile-group (weight DMAs are issued after the
second input group so they land just as L0 becomes runnable). Highway L1
carries one-unit-delayed fp8 cast + w3x slices, fp8 row-major transposes
per finished token-slice, and the fuse-gate weight drip, all placed in
engine slack. s1 and s2 come from one two-row GEMM (w1|w2 packed in the
stationary) plus a 1-row PE transpose, avoiding 32 stationary reloads.
Attention: S^T[j,i]=64*s3 fp8; exp on ACT (scale 1/64, bias s2[j]); the
per-column s1[i]+ab cancels in softmax so relu becomes a clamp against
th[i]=exp(-(s1+ab)); numerator/denominator are fp8 DoubleRow matmuls over
M^T j-tile pairs (per-pair ehq buffers); the denominator accumulates into
row 0 of the broadcast PSUM tile (saves a bank, enabling double-buffered
broadcasts), the next unit's threshold broadcast is prefetched before the
current unit's drains, and numerator banks drain to bf16 immediately with
normalization deferred off the critical path. Phase E preloads 64x biases
into PSUM via DVE (start=False chains) and casts attq one tile ahead.
"""

import numpy as np

B, L, U, H = 16, 1024, 512, 2
NCORES = 8
BPC = B // NCORES          # batches per core
N = BPC * L                # token columns per core
KU = U // 128              # 4  u-tiles
NT = N // 128              # 16 row-tiles per core
NS = N // 512              # 4  512-wide column slices per core
JT = L // 128              # 8  j-tiles per batch
IH = L // 512              # 2  i-halves per batch
WS = 64.0                  # fp8 weight pre-scale
WSI = 1.0 / WS


def build_nc():
    import concourse.bacc as bacc
    import concourse.tile as tile
    from concourse import mybir
    from concourse.masks import make_identity

    F32 = mybir.dt.float32
    F32R = mybir.dt.float32r
    BF16 = mybir.dt.bfloat16
    FP8 = mybir.dt.float8e4
    AF = mybir.ActivationFunctionType
    OP = mybir.AluOpType
    PM = mybir.MatmulPerfMode

    nc = bacc.Bacc("TRN2", target_bir_lowering=False, debug=False,
                   num_devices=NCORES)

    x_in = nc.dram_tensor("inputs", [BPC, L, U], F32, kind="ExternalInput").ap()
    tW = nc.dram_tensor("tW", [H, U, U], F32, kind="ExternalInput").ap()
    tb = nc.dram_tensor("tb", [H, U], F32, kind="ExternalInput").ap()
    cW = nc.dram_tensor("cW", [H, U, U], F32, kind="ExternalInput").ap()
    cb = nc.dram_tensor("cb", [H, U], F32, kind="ExternalInput").ap()
    aW = nc.dram_tensor("aW", [3 * U], F32, kind="ExternalInput").ap()
    ab = nc.dram_tensor("ab", [1], F32, kind="ExternalInput").ap()
    frW = nc.dram_tensor("frW", [2 * U, U], F32, kind="ExternalInput").ap()
    frb = nc.dram_tensor("frb", [U], F32, kind="ExternalInput").ap()
    ffW = nc.dram_tensor("ffW", [2 * U, U], F32, kind="ExternalInput").ap()
    ffb = nc.dram_tensor("ffb", [U], F32, kind="ExternalInput").ap()
    out = nc.dram_tensor("out", [BPC, L, U], F32, kind="ExternalOutput").ap()

    xv = x_in.flatten_outer_dims().rearrange("(t p) u -> t p u", p=128)
    outv = out.flatten_outer_dims().rearrange("(t p) u -> t p u", p=128)

    with tile.TileContext(nc) as tc:
        with tc.tile_pool(name="pers", bufs=1) as pers:
            # ---- persistent SBUF tensors ----
            xTh = pers.tile([128, KU, N], BF16, tag="xTh")    # inputs^T bf16
            xq = pers.tile([128, KU, N], FP8, tag="xq")       # inputs^T fp8
            x1T = pers.tile([128, KU, N], BF16, tag="x1T")
            x2T = pers.tile([128, KU, N], BF16, tag="x2T")
            x2q = pers.tile([128, KU, N], FP8, tag="x2q")
            w3xq = pers.tile([128, KU, N], FP8, tag="w3xq")   # (64*w3)*x^T
            attT = pers.tile([128, KU, N], BF16, tag="attT")
            attq = pers.tile([128, KU, N], FP8, tag="attq")
            xOq = pers.tile([128, NT, U], FP8, tag="xOq")     # x2 row-major
            tWq = pers.tile([128, KU, U], FP8, tag="tWq")     # 64*tW[0]
            cWq = pers.tile([128, KU, U], FP8, tag="cWq")     # 64*cW[0]
            tWh = pers.tile([128, KU, U], BF16, tag="tWh")    # tW[1]
            cWh = pers.tile([128, KU, U], BF16, tag="cWh")    # cW[1]
            ffWh = pers.tile([128, KU, U], BF16, tag="ffWh")  # 64*ffW[:U]
            frWh = pers.tile([128, KU, U], BF16, tag="frWh")  # 64*frW[:U]
            ffWq = pers.tile([128, KU, U], FP8, tag="ffWq")   # 64*ffW[U:]
            frWq = pers.tile([128, KU, U], FP8, tag="frWq")   # 64*frW[U:]
            tbsb = pers.tile([128, H, KU], F32, tag="tbsb")
            cbsb = pers.tile([128, H, KU], F32, tag="cbsb")
            awsb = pers.tile([128, 12], F32, tag="awsb")      # w1|w2|w3 cols
            # k-pair stride of DoubleRow operands must be 16B-aligned:
            # w1 and w2 are packed as two 64x-scaled columns of one
            # 16-wide stationary so s1 and s2 come from a single GEMM
            w12q = pers.tile([128, KU, 16], FP8, tag="w12q")
            aw3s = pers.tile([128, KU], F32, tag="aw3s")      # 64*w3
            ab_sb = pers.tile([1, 1], F32, tag="ab_sb")
            nab_sb = pers.tile([1, 1], F32, tag="nab_sb")
            ffb_h = pers.tile([1, U], BF16, tag="ffb_h")      # 64*ffb
            frb_h = pers.tile([1, U], BF16, tag="frb_h")      # 64*frb
            fbbc = pers.tile([128, U], F32, tag="fbbc")       # 64*ffb bcast
            rbbc = pers.tile([128, U], F32, tag="rbbc")       # 64*frb bcast
            thr = pers.tile([1, N], BF16, tag="thr")   # exp(-(s1+ab))
            s2r = pers.tile([1, N], BF16, tag="s2r")   # s2 row-major
            s2f = pers.tile([128, NT], F32, tag="s2f")
            ones_row = pers.tile([1, 128], BF16, tag="ones_row")
            onesq = pers.tile([128, 2, 16], FP8, tag="onesq")
            ident = pers.tile([128, 128], BF16, tag="ident")
            identq = pers.tile([128, 128], FP8, tag="identq")
            identf = pers.tile([128, 128], F32, tag="identf")

            nc.vector.memset(ones_row, 1.0)
            nc.vector.memset(onesq, 1.0)
            make_identity(nc, ident)
            make_identity(nc, identf)
            nc.vector.tensor_copy(identq, ident)

            fWv = ffW.rearrange("(k p) m -> k p m", p=128)
            rWv = frW.rearrange("(k p) m -> k p m", p=128)
            # x0-half chunks in bf16 (x64), att-half in fp8 (x64)
            fuse_bf = [(fWv, ffWh, k) for k in range(KU)] + \
                      [(rWv, frWh, k) for k in range(KU)]
            fuse_q = [(fWv, ffWq, k) for k in range(KU)] + \
                     [(rWv, frWq, k) for k in range(KU)]

            with tc.tile_pool(name="stg", bufs=6) as stg, \
                 tc.tile_pool(name="stgw", bufs=6) as stgw, \
                 tc.tile_pool(name="hws", bufs=4) as hws, \
                 tc.tile_pool(name="stgf", bufs=4) as stgf:

                def drip_bf(ci):
                    wv_, wdst_, k_ = fuse_bf[ci]
                    wsf = stgf.tile([128, U], F32, tag="wsf",
                                    name=f"wsf_b{ci}")
                    nc.sync.dma_start(wsf, wv_[k_])
                    nc.vector.tensor_scalar_mul(wdst_[:, k_, :], wsf, WS)

                def drip_q(ci):
                    wv_, wdst_, k_ = fuse_q[ci]
                    wsf = stgf.tile([128, U], F32, tag="wsf",
                                    name=f"wsf_q{ci}")
                    nc.sync.dma_start(wsf, wv_[KU + k_])
                    nc.scalar.activation(wdst_[:, k_, :], wsf, AF.Copy,
                                         scale=WS)

                with tc.tile_pool(name="hwp", bufs=2, space="PSUM") as hwp:

                    def hw_unit(l, t, m):
                        xin = xq if l == 0 else x1T
                        xinb = xTh if l == 0 else x1T
                        xout = x1T if l == 0 else x2T
                        nsl = slice(t * 512, (t + 1) * 512)
                        msl = slice(m * 128, (m + 1) * 128)
                        unit = t * KU + m
                        pt = hwp.tile([128, 512], F32, tag="pt")
                        pc = hwp.tile([128, 512], F32, tag="pc")
                        if l == 0:
                            for k2 in range(2):
                                ksl = slice(2 * k2, 2 * k2 + 2)
                                nc.tensor.matmul(
                                    pt, tWq[:, ksl, msl], xin[:, ksl, nsl],
                                    start=(k2 == 0), stop=(k2 == 1),
                                    perf_mode=PM.DoubleRow)
                            for k2 in range(2):
                                ksl = slice(2 * k2, 2 * k2 + 2)
                                nc.tensor.matmul(
                                    pc, cWq[:, ksl, msl], xin[:, ksl, nsl],
                                    start=(k2 == 0), stop=(k2 == 1),
                                    perf_mode=PM.DoubleRow)
                            sc = WSI
                        else:
                            for k in range(KU):
                                nc.tensor.matmul(
                                    pt, tWh[:, k, msl], xin[:, k, nsl],
                                    start=(k == 0), stop=(k == KU - 1))
                            for k in range(KU):
                                nc.tensor.matmul(
                                    pc, cWh[:, k, msl], xin[:, k, nsl],
                                    start=(k == 0), stop=(k == KU - 1))
                            sc = 1.0
                        th = hws.tile([128, 512], BF16, tag="th")
                        ch = hws.tile([128, 512], BF16, tag="ch")
                        nc.scalar.activation(
                            th, pt, AF.Relu, bias=tbsb[:, l, m:m + 1],
                            scale=sc)
                        nc.scalar.activation(
                            ch, pc, AF.Sigmoid, bias=cbsb[:, l, m:m + 1],
                            scale=sc)
                        dh = hws.tile([128, 512], BF16, tag="dh")
                        nc.vector.tensor_tensor(
                            dh, th, xinb[:, m, nsl], op=OP.subtract)
                        mh = hws.tile([128, 512], BF16, tag="mh")
                        nc.vector.tensor_tensor(mh, ch, dh, op=OP.mult)
                        nc.gpsimd.tensor_tensor(
                            xout[:, m, nsl], xinb[:, m, nsl], mh, op=OP.add)
                        if l == 0:
                            if unit % 2 == 0:
                                drip_bf(unit // 2)
                        else:
                            if unit % 2 == 0:
                                drip_q(unit // 2)

                    # ===== Phase A + L0 + L1 software-pipelined ============
                    # psum: hwp 4 + ptA 2 + pcp 2 = 8 banks
                    with tc.tile_pool(name="ptA", bufs=1,
                                      space="PSUM") as ptA, \
                         tc.tile_pool(name="pcp", bufs=2,
                                      space="PSUM") as pcp:

                        def l1_delayed(t, m):
                            nsl = slice(t * 512, (t + 1) * 512)
                            nc.vector.tensor_copy(x2q[:, m, nsl],
                                                  x2T[:, m, nsl])
                            nc.scalar.activation(
                                w3xq[:, m, nsl], x2q[:, m, nsl], AF.Copy,
                                scale=aw3s[:, m:m + 1])

                        def xo_jt(jt):
                            # bf16 transpose straight from x2T (no wait on
                            # the fp8 cast); DVE drains to fp8
                            ptr = pcp.tile([128, KU, 128], BF16,
                                           tag="ptr", name=f"ptr_{jt}")
                            for k in range(KU):
                                nc.tensor.transpose(
                                    ptr[:, k, :],
                                    x2T[:, k, jt * 128:(jt + 1) * 128],
                                    ident)
                            nc.vector.tensor_copy(xOq[:, jt, :], ptr)

                        def l1_unit(t, m):
                            hw_unit(1, t, m)
                            unit = t * KU + m
                            if unit >= 1:
                                pm_ = (unit - 1) % 4
                                pt_ = (unit - 1) // 4
                                l1_delayed(pt_, pm_)
                            if t >= 1:
                                xo_jt(4 * (t - 1) + m)

                        def transposes(tg):
                            xss = []
                            for tt in range(4):
                                t = tg * 4 + tt
                                xs = stg.tile([128, U], F32, tag="xs",
                                              name=f"xs_{t}")
                                nc.sync.dma_start(xs[:, 0:256],
                                                  xv[t][:, 0:256])
                                nc.sync.dma_start(xs[:, 256:512],
                                                  xv[t][:, 256:512])
                                xss.append(xs)
                            tsl = slice(tg * 512, (tg + 1) * 512)
                            for k in range(KU):
                                ptk = ptA.tile([128, 512], F32,
                                               tag=f"ptk{k % 2}",
                                               name=f"ptk_{tg}_{k}")
                                for tt in range(4):
                                    nc.tensor.transpose(
                                        ptk[:, tt * 128:(tt + 1) * 128],
                                        xss[tt][:, k * 128:(k + 1) * 128],
                                        identf)
                                nc.vector.tensor_copy(xTh[:, k, tsl], ptk)
                                if k % 2 == 0:
                                    nc.vector.tensor_copy(xq[:, k, tsl],
                                                          ptk)
                                else:
                                    nc.scalar.activation(xq[:, k, tsl],
                                                         ptk, AF.Copy)

                        def emit_hw_weights():
                            for l in range(H):
                                for wi, wsrc in ((0, tW), (1, cW)):
                                    wdst = ((tWq, cWq), (tWh, cWh))[l][wi]
                                    wv = wsrc[l].rearrange(
                                        "(k p) m -> k p m", p=128)
                                    for k in range(KU):
                                        ws = stgw.tile(
                                            [128, U], F32, tag="ws",
                                            name=f"ws_{l}_{wi}_{k}")
                                        nc.sync.dma_start(ws[:, 0:256],
                                                          wv[k][:, 0:256])
                                        nc.sync.dma_start(ws[:, 256:512],
                                                          wv[k][:, 256:512])
                                        if l == 0:
                                            if k % 2 == 0:
                                                nc.vector.tensor_scalar_mul(
                                                    wdst[:, k, :], ws, WS)
                                            else:
                                                nc.scalar.activation(
                                                    wdst[:, k, :], ws,
                                                    AF.Copy, scale=WS)
                                        else:
                                            if k % 2 == 0:
                                                nc.vector.tensor_copy(
                                                    wdst[:, k, :], ws)
                                            else:
                                                nc.scalar.copy(
                                                    wdst[:, k, :], ws)

                        def emit_smalls():
                            nc.sync.dma_start(
                                tbsb, tb.rearrange("l (m p) -> p l m",
                                                   p=128))
                            nc.sync.dma_start(
                                cbsb, cb.rearrange("l (m p) -> p l m",
                                                   p=128))
                            nc.sync.dma_start(
                                awsb,
                                aW.rearrange("(w m p) -> p (w m)",
                                             p=128, w=3))
                            nc.vector.tensor_scalar_mul(w12q[:, :, 0],
                                                        awsb[:, 0:KU], WS)
                            nc.vector.tensor_scalar_mul(w12q[:, :, 8],
                                                        awsb[:, KU:2 * KU],
                                                        WS)
                            nc.vector.tensor_scalar_mul(
                                aw3s, awsb[:, 2 * KU:3 * KU], WS)
                            nc.sync.dma_start(ab_sb, ab[None, :])
                            nc.scalar.mul(nab_sb, ab_sb, -1.0)
                            fb = stg.tile([1, U], F32, tag="fb")
                            nc.sync.dma_start(fb, ffb[None, :])
                            nc.vector.tensor_scalar_mul(ffb_h, fb, WS)
                            fb2 = stg.tile([1, U], F32, tag="fb")
                            nc.sync.dma_start(fb2, frb[None, :])
                            nc.vector.tensor_scalar_mul(frb_h, fb2, WS)

                        # input DMAs first (first-tile latency gates the
                        # transposes); params + weights queue behind
                        warm = [ptA.tile([128, 512], F32, tag=f"ptk{k}",
                                         name=f"warm_{k}")
                                for k in range(2)]
                        for i in range(40):
                            nc.tensor.matmul(warm[i % 2][:, 0:128], ident,
                                             ident, start=True, stop=True)
                        transposes(0)
                        transposes(1)
                        emit_smalls()
                        emit_hw_weights()
                        for m in range(KU):
                            hw_unit(0, 0, m)
                        transposes(2)
                        for m in range(KU):
                            hw_unit(0, 1, m)
                        for m in range(KU):
                            l1_unit(0, m)
                        transposes(3)
                        for m in range(KU):
                            hw_unit(0, 2, m)
                        for m in range(KU):
                            l1_unit(1, m)
                        for m in range(KU):
                            hw_unit(0, 3, m)
                        for m in range(KU):
                            l1_unit(2, m)
                        for m in range(KU):
                            l1_unit(3, m)
                        l1_delayed(3, 3)
                        for jt in range(12, 16):
                            xo_jt(jt)

                # ============= mini-C: s1/s2 + bias broadcasts =============
                with tc.tile_pool(name="pcp1", bufs=1, space="PSUM") as pcp1:
                    for t in range(NS):
                        tsl = slice(t * 512, (t + 1) * 512)
                        ps1 = pcp1.tile([1, 512], F32, tag="ps1",
                                        name=f"ps1_{t}")
                        for k2 in range(2):
                            ksl = slice(2 * k2, 2 * k2 + 2)
                            nc.tensor.matmul(
                                ps1, w12q[:, ksl, 0:1], x2q[:, ksl, tsl],
                                start=(k2 == 0), stop=(k2 == 1),
                                perf_mode=PM.DoubleRow)
                        nc.scalar.activation(thr[:, tsl], ps1,
                                             AF.Exp, bias=nab_sb, scale=-WSI)
                        ps2 = pcp1.tile([1, 512], F32, tag="ps2",
                                        name=f"ps2_{t}")
                        for k2 in range(2):
                            ksl = slice(2 * k2, 2 * k2 + 2)
                            nc.tensor.matmul(
                                ps2, w12q[:, ksl, 8:9], x2q[:, ksl, tsl],
                                start=(k2 == 0), stop=(k2 == 1),
                                perf_mode=PM.DoubleRow)
                        nc.vector.tensor_scalar_mul(s2r[:, tsl], ps2, WSI)
                    s2tp = pcp1.tile([128, NT, 2], BF16, tag="s2tp")
                    for jt in range(NT):
                        nc.tensor.transpose(
                            s2tp[:, jt, 0:1],
                            s2r[:, jt * 128:(jt + 1) * 128],
                            ones_row[:, 0:1])
                    nc.vector.tensor_copy(s2f, s2tp[:, :, 0])
                    pfb = pcp1.tile([128, U], F32, tag="pfb")
                    nc.tensor.matmul(pfb, ones_row, ffb_h,
                                     start=True, stop=True)
                    nc.vector.tensor_copy(fbbc, pfb)
                    prb = pcp1.tile([128, U], F32, tag="prb")
                    nc.tensor.matmul(prb, ones_row, frb_h,
                                     start=True, stop=True)
                    nc.vector.tensor_copy(rbbc, prb)

            # ============= Phase D: pairwise softmax attention (fp8) =======
            units = [(b, h) for b in range(BPC) for h in range(IH)]
            with tc.tile_pool(name="pdn", bufs=4, space="PSUM") as pdn, \
                 tc.tile_pool(name="pds", bufs=2, space="PSUM") as pds, \
                 tc.tile_pool(name="pbc", bufs=2, space="PSUM") as pbc, \
                 tc.tile_pool(name="dsb", bufs=4) as dsb:

                def emit_bcast(u):
                    # threshold broadcast for unit u; the denominator later
                    # accumulates into row 0 of the same psum tile
                    b, h = units[u]
                    isl = slice(b * L + h * 512, b * L + (h + 1) * 512)
                    pb = pbc.tile([128, 512], F32, tag="pb",
                                  name=f"pb1_{u}")
                    nc.tensor.matmul(pb, ones_row, thr[:, isl],
                                     start=True, stop=True)
                    tb_ = dsb.tile([128, 512], BF16, tag="thbc",
                                   name=f"thbc_{u}")
                    nc.scalar.copy(tb_, pb)
                    return pb, tb_

                nxt = emit_bcast(0)
                for u, (b, h) in enumerate(units):
                    pb1, thbc = nxt
                    pr = pb1[0:1, :]
                    isl = slice(b * L + h * 512, b * L + (h + 1) * 512)
                    pn = [pdn.tile([128, 512], F32, tag="pn",
                                   name=f"pn_{u}_{du}")
                          for du in range(KU)]
                    for jt in range(JT):
                        jg = b * JT + jt
                        jsl = slice(b * L + jt * 128, b * L + (jt + 1) * 128)
                        if jt % 2 == 0:
                            ehq = dsb.tile([128, 2, 512], FP8, tag="ehq",
                                           name=f"ehq_{u}_{jt // 2}")
                        ps = pds.tile([128, 512], F32, tag="ps")
                        for k2 in range(2):
                            ksl = slice(2 * k2, 2 * k2 + 2)
                            nc.tensor.matmul(ps, w3xq[:, ksl, jsl],
                                             x2q[:, ksl, isl],
                                             start=(k2 == 0), stop=(k2 == 1),
                                             perf_mode=PM.DoubleRow)
                        eh = dsb.tile([128, 512], BF16, tag="eh")
                        nc.scalar.activation(eh, ps, AF.Exp,
                                             bias=s2f[:, jg:jg + 1],
                                             scale=WSI)
                        nc.vector.tensor_tensor(ehq[:, jt % 2, :],
                                                eh, thbc, op=OP.max)
                        if jt % 2 == 1:
                            jg0 = jg - 1
                            pair = jt // 2
                            for du in range(KU):
                                nc.tensor.matmul(
                                    pn[du],
                                    xOq[:, jg0:jg0 + 2,
                                        du * 128:(du + 1) * 128],
                                    ehq,
                                    start=(pair == 0),
                                    stop=(pair == JT // 2 - 1),
                                    perf_mode=PM.DoubleRow)
                            nc.tensor.matmul(pr, onesq[:, :, 0:1], ehq,
                                             start=(pair == 0),
                                             stop=(pair == JT // 2 - 1),
                                             perf_mode=PM.DoubleRow)
                    rec = dsb.tile([1, 512], F32, tag="rec")
                    nc.vector.reciprocal_approx_fast(rec, pr)
                    rech = dsb.tile([1, 512], BF16, tag="rech")
                    nc.vector.tensor_copy(rech, rec)
                    if u + 1 < len(units):
                        nxt = emit_bcast(u + 1)
                    # drain numerator banks promptly; normalize off-path
                    pnh = [dsb.tile([128, 512], BF16, tag="pnh",
                                    name=f"pnh_{u}_{du}")
                           for du in range(KU)]
                    for du in range(KU):
                        if du % 2 == 0:
                            nc.scalar.copy(pnh[du], pn[du])
                        else:
                            nc.vector.tensor_copy(pnh[du], pn[du])
                    rbc = dsb.tile([128, 512], BF16, tag="rbc")
                    pb2 = pbc.tile([128, 512], F32, tag="pb",
                                   name=f"pb2_{u}")
                    nc.tensor.matmul(pb2, ones_row, rech,
                                     start=True, stop=True)
                    nc.scalar.copy(rbc, pb2)
                    for du in (0, 3):
                        nc.vector.tensor_tensor(
                            attT[:, du, isl], pnh[du], rbc, op=OP.mult)
                    for du in (1, 2):
                        nc.gpsimd.tensor_tensor(
                            attT[:, du, isl], pnh[du], rbc, op=OP.mult)

            # ==== Phase E: fuse gates (x0 bf16 + att fp8, both x64) ========
            with tc.tile_pool(name="pep", bufs=2, space="PSUM") as pep, \
                 tc.tile_pool(name="esb", bufs=3) as esb:
                nc.vector.tensor_copy(attq[:, :, 0:128], attT[:, :, 0:128])
                for mt in range(NT):
                    msl = slice(mt * 128, (mt + 1) * 128)
                    if mt + 1 < NT:
                        nsl = slice((mt + 1) * 128, (mt + 2) * 128)
                        nc.vector.tensor_copy(attq[:, :, nsl],
                                              attT[:, :, nsl])
                    x0t = esb.tile([128, U], F32, tag="x0t")
                    nc.sync.dma_start(x0t, xv[mt])
                    pz = pep.tile([128, 512], F32, tag="pz")
                    pr2 = pep.tile([128, 512], F32, tag="pr2")
                    nc.vector.tensor_copy(pz, fbbc)
                    nc.vector.tensor_copy(pr2, rbbc)
                    for k in range(KU):
                        lhsT = xTh[:, k, msl]
                        nc.tensor.matmul(pz, lhsT, ffWh[:, k, :],
                                         start=False, stop=False)
                        nc.tensor.matmul(pr2, lhsT, frWh[:, k, :],
                                         start=False, stop=False)
                    for i in range(KU // 2):
                        ksl = slice(2 * i, 2 * i + 2)
                        lhsT = attq[:, ksl, msl]
                        nc.tensor.matmul(pz, lhsT, ffWq[:, ksl, :],
                                         start=False,
                                         stop=(i == KU // 2 - 1),
                                         perf_mode=PM.DoubleRow)
                        nc.tensor.matmul(pr2, lhsT, frWq[:, ksl, :],
                                         start=False,
                                         stop=(i == KU // 2 - 1),
                                         perf_mode=PM.DoubleRow)
                    zh = esb.tile([128, U], BF16, tag="zh")
                    rh = esb.tile([128, U], BF16, tag="rh")
                    q = esb.tile([128, U], F32, tag="q")
                    p2 = esb.tile([128, U], F32, tag="p2")
                    ot = esb.tile([128, U], F32, tag="ot")
                    if mt == NT - 1:
                        # last unit sets the kernel tail: shorten its
                        # serial chain by splitting across engines
                        hU = U // 2
                        nc.scalar.activation(zh, pz, AF.Sigmoid, scale=WSI)
                        nc.scalar.square(q, zh)
                        nc.scalar.activation(rh, pr2, AF.Sigmoid, scale=WSI)
                        nc.vector.tensor_tensor(p2[:, :hU], rh[:, :hU],
                                                x0t[:, :hU], op=OP.mult)
                        nc.gpsimd.tensor_tensor(p2[:, hU:], rh[:, hU:],
                                                x0t[:, hU:], op=OP.mult)
                        nc.vector.tensor_tensor(ot[:, :hU], q[:, :hU],
                                                p2[:, :hU], op=OP.add)
                        nc.gpsimd.tensor_tensor(ot[:, hU:], q[:, hU:],
                                                p2[:, hU:], op=OP.add)
                    else:
                        nc.scalar.activation(zh, pz, AF.Sigmoid, scale=WSI)
                        nc.scalar.activation(rh, pr2, AF.Sigmoid, scale=WSI)
                        nc.gpsimd.tensor_tensor(q, zh, zh, op=OP.mult)
                        if mt % 2 == 0:
                            nc.vector.tensor_tensor(p2, rh, x0t, op=OP.mult)
                            nc.vector.tensor_tensor(ot, q, p2, op=OP.add)
                        else:
                            nc.gpsimd.tensor_tensor(p2, rh, x0t, op=OP.mult)
                            nc.gpsimd.tensor_tensor(ot, q, p2, op=OP.add)
                    nc.sync.dma_start(outv[mt], ot)

    nc.compile()
    return nc


_NC_CACHE = None


def _get_nc():
    global _NC_CACHE
    if _NC_CACHE is None:
        _NC_CACHE = build_nc()
    return _NC_CACHE


def kernel(**inputs) -> np.ndarray:
    from concourse.bass_utils import run_bass_kernel_spmd

    nc = _get_nc()
    full = {k: np.ascontiguousarray(np.asarray(v, dtype=np.float32))
            for k, v in inputs.items()}
    in_maps = []
    for c in range(NCORES):
        m = dict(full)
        m["inputs"] = np.ascontiguousarray(
            full["inputs"][c * BPC:(c + 1) * BPC])
        in_maps.append(m)
    res = run_bass_kernel_spmd(nc, in_maps, core_ids=list(range(NCORES)))
    return np.concatenate([res.results[c]["out"] for c in range(NCORES)],
                          axis=0)
